# revision 8
# baseline (speedup 1.0000x reference)
"""LocallyConnected2d Bass kernel for 8 Trainium2 NeuronCores.

Problem (hardcoded): x[16,32,64,64] f32, weight[64,64,32,32,3,3] f32,
bias[32,64,64] f32 -> out[16,32,64,64] f32.  stride=1, pad=1, dil=1.

Sharding: outH split across 8 cores (8 rows each).  Per core, per output
row h: 64 w-positions x 3 kernel-rows of matmuls [K<=97,M=32]x[K,N=16]
accumulated in PSUM.  K = (kernel-col j)*32 + inC c, with a 97th "ones"
row carrying the bias.

Traffic optimizations vs the bf16 baseline:
  - weights (the dominant stream) stored fp8 e3m4, scaled by 2^8 on host
    (output descaled by 2^-8 on host - exact).  Halves weight HBM bytes.
  - x is DMAed once (unreplicated, [32c, 10hh, 66wp*16b] bf16) into
    partitions 0..31; the kernel-column-shifted copies for partition
    groups 1,2 (j=1,2) are made on-chip by DVE partition-shifted copies
    (4x perf mode), cutting x HBM bytes 3x.
  - one weight DMA per output row (row 7 in two halves to shorten the
    dependent tail); out DMAs issued on the sync ring after all weight
    DMAs so weights win the DMA-engine queue.

w-positions are processed in quads: position w = q*4+g is computed by a
matmul col-tiled to column group g (tile_position=(0,32g)), so the four
LDWEIGHTS+MATMUL streams of a quad run concurrently in the PE array.
PSUM tile is [128 = 4w x 32o, 16 quads x 16b] per output row.
"""

import numpy as np
import ml_dtypes

B, C, H, W = 16, 32, 64, 64
OC = 32
KH = KW = 3
NCORES = 8
RPC = H // NCORES  # rows per core = 8
NQ = 4  # quad size (PE col groups)
WSCALE = 2.0**8  # weight scale into fp8e3 range (max 15.08 < 15.5)
WP = W + 2  # padded width positions per row

BF16 = ml_dtypes.bfloat16
F8E3 = ml_dtypes.float8_e3m4

# x tile chunking by padded row hh: chunk -> (hh0, hh1)
XCHUNKS = [(0, 3), (3, 7), (7, 10)]

_cache = {}


def _build_nc():
    import concourse.bass as bass
    import concourse.tile as tile
    from concourse import bacc, mybir

    nc = bacc.Bacc(
        "TRN2", target_bir_lowering=False, debug=False, num_devices=NCORES
    )
    f32 = mybir.dt.float32
    f16 = mybir.dt.float16
    bf16 = mybir.dt.bfloat16
    f8e3 = mybir.dt.float8e3

    # xs: [33, 10, 66*16] bf16.  Partition c<32 holds x[c, hh, wp, b]
    # (hh = local padded row 0..9, wp = padded col 0..65, b = batch);
    # partition 32 is all-ones (bias row source).
    xs = nc.dram_tensor("xs", (33, 10, WP * B), bf16, kind="ExternalInput")
    # wt: [8, 97, 64*3*32] f8e3; [h, j*32+c, (w*3+ik)*32+o] scaled by 2^8;
    # row 96 holds bias*2^8 at ik==2 slots, zeros elsewhere.
    wt = nc.dram_tensor(
        "wt", (RPC, 97, W * KH * OC), f8e3, kind="ExternalInput"
    )
    # out: [8, 128, 16*16] f16 = 2^8 * out[h, g*32+o, q*16+b] with w = q*4+g
    out = nc.dram_tensor(
        "out", (RPC, 4 * OC, (W // NQ) * B), f16, kind="ExternalOutput"
    )

    with tile.TileContext(nc) as tc:
        with (
            tc.tile_pool(name="xpool", bufs=1) as xpool,
            tc.tile_pool(name="wpool", bufs=9) as wpool,
            tc.tile_pool(name="opool", bufs=4) as opool,
            tc.tile_pool(name="psum", bufs=3, space="PSUM") as ppool,
            tc.tile_pool(name="psum7", bufs=1, space="PSUM") as ppool7,
        ):
            # Per-chunk x tiles [97, rows, WP*16].  DMA x once into
            # partitions 0..31, ones row into 96 (scalar ring); DVE makes
            # the j=1,2 column-shifted copies into partitions 32..95.
            # x DMAs ride the sync ring interleaved ahead of the weight
            # rows so the early DMA-engine slots go x0, w0, x1, x2, w1...
            xtiles = []
            for ci, (h0, h1) in enumerate(XCHUNKS):
                r = h1 - h0
                t = xpool.tile([97, r, WP * B], bf16, tag=f"x{ci}")
                xtiles.append(t)

            def load_xchunk(ci):
                h0, h1 = XCHUNKS[ci]
                t = xtiles[ci]
                nc.sync.dma_start(t[0:32], xs[0:32, h0:h1])
                nc.scalar.dma_start(
                    t[96:97, :, 0 : W * B], xs[32:33, h0:h1, 0 : W * B]
                )
                for j in (1, 2):
                    nc.vector.tensor_copy(
                        t[32 * j : 32 * (j + 1), :, 0 : W * B],
                        t[0:32, :, j * B : j * B + W * B],
                    )

            def xslice(hh, w, k):
                for (h0, h1), t in zip(XCHUNKS, xtiles):
                    if h0 <= hh < h1:
                        return t[0:k, hh - h0, w * B : (w + 1) * B]
                raise AssertionError

            # Weight DMAs on the sync ring: rows 0..6 whole; row 7 in
            # half + quarter + quarter pieces so the final dependent
            # compute+copy+out chain after the last weight byte is short.
            # Interleave the x chunk loads ahead of the early weight rows.
            load_xchunk(0)
            wtiles = []
            for h in range(RPC - 1):
                if h == 0:
                    pass
                wti = wpool.tile([97, W * KH * OC], f8e3)
                nc.sync.dma_start(wti[:], wt[h])
                wtiles.append(wti)
                if h == 0:
                    load_xchunk(1)
                    load_xchunk(2)
            # row-7 pieces: quads [0,8), [8,12), [12,16)
            W7Q = [(0, 8), (8, 12), (12, 16)]
            w7tiles = []
            for pi, (q0, q1) in enumerate(W7Q):
                cols = (q1 - q0) * NQ * KH * OC
                t = wpool.tile([97, cols], f8e3, tag=f"w7{pi}")
                nc.sync.dma_start(
                    t[:], wt[RPC - 1, :, q0 * NQ * KH * OC :][:, 0:cols]
                )
                w7tiles.append(t)

            def wslice(h, w, ik, k):
                if h < RPC - 1:
                    return wtiles[h][0:k, (w * 3 + ik) * 32 :][:, 0:32]
                for (q0, q1), t in zip(W7Q, w7tiles):
                    if q0 * NQ <= w < q1 * NQ:
                        wl = w - q0 * NQ
                        return t[0:k, (wl * 3 + ik) * 32 :][:, 0:32]
                raise AssertionError

            outs = []  # (dram slice, sbuf tile) deferred out DMAs

            def do_row(h, pt, q0, q1, osl):
                for q in range(q0, q1):
                    for g in range(NQ):
                        w = q * NQ + g
                        for ik in range(KH):
                            k = 97 if ik == 2 else 96
                            nc.tensor.matmul(
                                pt[
                                    32 * g : 32 * (g + 1),
                                    (q - q0) * B : (q - q0 + 1) * B,
                                ],
                                wslice(h, w, ik, k),
                                xslice(h + ik, w, k),
                                start=(ik == 0),
                                stop=(ik == 2),
                                tile_position=(0, 32 * g),
                            )
                ot = opool.tile([4 * OC, (q1 - q0) * B], f16)
                nc.vector.tensor_copy(ot[:], pt[:])
                outs.append((osl, ot))

            NQW = W // NQ  # 16 quads per row
            for h in range(RPC - 1):
                pt = ppool.tile([4 * OC, NQW * B], f32)
                do_row(h, pt, 0, NQW, out[h])
            # row 7: one PSUM piece per weight piece
            for pi, (q0, q1) in enumerate(W7Q):
                pt = ppool7.tile([4 * OC, (q1 - q0) * B], f32, tag=f"p7{pi}")
                do_row(
                    RPC - 1, pt, q0, q1, out[RPC - 1, :, q0 * B : q1 * B]
                )

            # out DMAs on the sync ring, issued after all weight DMAs so
            # weight transfers win the DMA-engine queue.
            for osl, ot in outs:
                nc.sync.dma_start(osl, ot[:])
    nc.compile()
    return nc


def _prep_inputs(x, weight, bias):
    """Host-side shard + layout prep.  Returns list of 8 per-core dicts."""
    # padded x, transposed to [c, hh, wp, b]
    xp = np.zeros((C, H + 2, W + 2, B), dtype=BF16)
    xp[:, 1 : H + 1, 1 : W + 1, :] = np.ascontiguousarray(
        x.transpose(1, 2, 3, 0)
    ).astype(BF16)

    # weight -> [h, j, c, w, ik, o], scaled into fp8e3 range
    wtr = np.ascontiguousarray(
        weight.transpose(0, 5, 3, 1, 4, 2) * np.float32(WSCALE)
    ).astype(F8E3)
    wtr = wtr.reshape(H, 96, W, KH, OC)
    btr = (bias.transpose(1, 2, 0) * np.float32(WSCALE)).astype(F8E3)  # [h,w,o]

    in_maps = []
    for i in range(NCORES):
        h0 = i * RPC
        xcore = np.ones((33, RPC + 2, WP, B), dtype=BF16)
        xcore[0:32] = xp[:, h0 : h0 + RPC + 2, :, :]

        wcore = np.zeros((RPC, 97, W, KH, OC), dtype=F8E3)
        wcore[:, 0:96] = wtr[h0 : h0 + RPC]
        wcore[:, 96, :, 2, :] = btr[h0 : h0 + RPC]  # bias via ones-row, ik==2

        in_maps.append(
            {
                "xs": np.ascontiguousarray(xcore.reshape(33, RPC + 2, WP * B)),
                "wt": np.ascontiguousarray(
                    wcore.reshape(RPC, 97, W * KH * OC)
                ),
            }
        )
    return in_maps


def _run(in_maps, trace=False, tmpdir=None):
    from concourse.bass_utils import run_bass_kernel_spmd

    if "nc" not in _cache:
        _cache["nc"] = _build_nc()
    return run_bass_kernel_spmd(
        _cache["nc"], in_maps, list(range(NCORES)), trace=trace, tmpdir=tmpdir
    )


def _assemble(results):
    out = np.empty((B, OC, H, W), dtype=np.float32)
    inv = np.float32(1.0 / WSCALE)
    for i in range(NCORES):
        # res: [h, g*32+o, q*16+b], w = q*4+g
        res = (
            results[i]["out"].astype(np.float32).reshape(RPC, NQ, OC, W // NQ, B)
            * inv
        )
        # -> out[b, o, h, q*4+g]
        out[:, :, i * RPC : (i + 1) * RPC, :] = res.transpose(
            4, 2, 0, 3, 1
        ).reshape(B, OC, RPC, W)
    return out


def kernel(x, weight, bias):
    x = np.asarray(x)
    weight = np.asarray(weight)
    bias = np.asarray(bias)
    in_maps = _prep_inputs(x, weight, bias)
    results = _run(in_maps).results
    return _assemble(results)


# revision 14
# speedup vs baseline: 1.0521x; 1.0521x over previous
"""LocallyConnected2d Bass kernel for 8 Trainium2 NeuronCores.

Problem (hardcoded): x[16,32,64,64] f32, weight[64,64,32,32,3,3] f32,
bias[32,64,64] f32 -> out[16,32,64,64] f32.  stride=1, pad=1, dil=1.

Sharding: outH split across 8 cores (8 rows each).  Per core, per output
row h: 64 w-positions x 3 kernel-rows of matmuls [K<=97,M=32]x[K,N=16]
accumulated in PSUM.  K = (kernel-col j)*32 + inC c, with a 97th "ones"
row carrying the bias.

Traffic optimizations vs the bf16 baseline:
  - weights (the dominant stream) stored fp8 e3m4, scaled by 2^8 on host
    (output descaled by 2^-8 on host - exact).  Halves weight HBM bytes.
  - x is DMAed once (unreplicated, [32c, 10hh, 66wp*16b] bf16) into
    partitions 0..31; the kernel-column-shifted copies for partition
    groups 1,2 (j=1,2) are made on-chip by DVE partition-shifted copies
    (4x perf mode), cutting x HBM bytes 3x.
  - one weight DMA per output row (row 7 in two halves to shorten the
    dependent tail); out DMAs issued on the sync ring after all weight
    DMAs so weights win the DMA-engine queue.

w-positions are processed in quads: position w = q*4+g is computed by a
matmul col-tiled to column group g (tile_position=(0,32g)), so the four
LDWEIGHTS+MATMUL streams of a quad run concurrently in the PE array.
PSUM tile is [128 = 4w x 32o, 16 quads x 16b] per output row.
"""

import numpy as np
import ml_dtypes

B, C, H, W = 16, 32, 64, 64
OC = 32
KH = KW = 3
NCORES = 8
RPC = H // NCORES  # rows per core = 8
NQ = 4  # quad size (PE col groups)
WSCALE = 2.0**8  # weight scale into fp8e3 range (max 15.08 < 15.5)
WP = W + 2  # padded width positions per row

BF16 = ml_dtypes.bfloat16
F8E3 = ml_dtypes.float8_e3m4

# x tile chunking by padded row hh: chunk -> (hh0, hh1)
XCHUNKS = [(0, 3), (3, 7), (7, 10)]

_cache = {}


def _build_nc():
    import concourse.bass as bass
    import concourse.tile as tile
    from concourse import bacc, mybir

    nc = bacc.Bacc(
        "TRN2", target_bir_lowering=False, debug=False, num_devices=NCORES
    )
    f32 = mybir.dt.float32
    f16 = mybir.dt.float16
    bf16 = mybir.dt.bfloat16
    f8e3 = mybir.dt.float8e3

    # xs: [33, 10, 66*16] bf16.  Partition c<32 holds x[c, hh, wp, b]
    # (hh = local padded row 0..9, wp = padded col 0..65, b = batch);
    # partition 32 is all-ones (bias row source).
    xs = nc.dram_tensor("xs", (33, 10, WP * B), bf16, kind="ExternalInput")
    # wt: [8, 97, 64*3*32] f8e3; [h, j*32+c, (w*3+ik)*32+o] scaled by 2^8;
    # row 96 holds bias*2^8 at ik==2 slots, zeros elsewhere.
    wt = nc.dram_tensor(
        "wt", (RPC, 97, W * KH * OC), f8e3, kind="ExternalInput"
    )
    # out: [8, 128, 16*16] f16 = 2^8 * out[h, g*32+o, q*16+b] with w = q*4+g
    out = nc.dram_tensor(
        "out", (RPC, 4 * OC, (W // NQ) * B), f16, kind="ExternalOutput"
    )

    with tile.TileContext(nc) as tc:
        with (
            tc.tile_pool(name="xpool", bufs=1) as xpool,
            tc.tile_pool(name="wpool", bufs=1) as wpool,
            tc.tile_pool(name="opool", bufs=1) as opool,
            tc.tile_pool(name="psum", bufs=3, space="PSUM") as ppool,
        ):
            # Per-chunk x tiles [97, rows, WP*16].  DMA x once into
            # partitions 0..31, ones row into 96; DVE makes the j=1,2
            # column-shifted copies into partitions 32..95.  Everything
            # rides the single sync ring in a hand-ordered sequence so
            # the serialized DMA engines see x0,x1,x2,w0,ones,w1,... and
            # the HWDGE descriptor-gen (~0.63us/DMA) stays ahead of the
            # transfer stream.
            xtiles = []
            for ci, (h0, h1) in enumerate(XCHUNKS):
                r = h1 - h0
                t = xpool.tile([97, r, WP * B], bf16, tag=f"x{ci}")
                nc.sync.dma_start(t[0:32], xs[0:32, h0:h1])
                xtiles.append(t)

            def xcopies(ci):
                h0, h1 = XCHUNKS[ci]
                t = xtiles[ci]
                for j in (1, 2):
                    nc.vector.tensor_copy(
                        t[32 * j : 32 * (j + 1), :, 0 : W * B],
                        t[0:32, :, j * B : j * B + W * B],
                    )

            def load_ones(ci):
                h0, h1 = XCHUNKS[ci]
                t = xtiles[ci]
                nc.sync.dma_start(
                    t[96:97, :, 0 : W * B], xs[32:33, h0:h1, 0 : W * B]
                )

            def xslice(hh, w, k):
                for (h0, h1), t in zip(XCHUNKS, xtiles):
                    if h0 <= hh < h1:
                        return t[0:k, hh - h0, w * B : (w + 1) * B]
                raise AssertionError

            # Weight DMAs, one tile per row, loaded in quad-range pieces:
            # rows 0..3 whole, rows 4..6 in halves, row 7 in quarters.
            # Finer pieces toward the end shorten "weight bytes not yet
            # arrived when their dependent compute remains" without
            # letting the ~0.63us/DMA HWDGE cost outrun the transfers.
            WPIECES = {h: [(0, 16)] for h in range(4)}
            WPIECES.update({h: [(0, 8), (8, 16)] for h in (4, 5, 6)})
            WPIECES[7] = [(0, 4), (4, 8), (8, 12), (12, 16)]
            wtiles = {h: [] for h in range(RPC)}  # [(q0, q1, tile), ...]

            def load_w(h):
                for pi, (q0, q1) in enumerate(WPIECES[h]):
                    c0, c1 = q0 * NQ * KH * OC, q1 * NQ * KH * OC
                    t = wpool.tile([97, c1 - c0], f8e3, tag=f"w{h}_{pi}")
                    nc.sync.dma_start(t[:], wt[h, :, c0:c1])
                    wtiles[h].append((q0 * NQ, q1 * NQ, t))

            load_w(0)
            for ci in range(3):
                load_ones(ci)
            for h in range(1, RPC):
                load_w(h)

            def wslice(h, w, ik, k):
                for w0, w1, t in wtiles[h]:
                    if w0 <= w < w1:
                        return t[0:k, ((w - w0) * 3 + ik) * 32 :][:, 0:32]
                raise AssertionError

            # x replication copies for chunks 0,1 ahead of all PSUM
            # copies in the DVE queue; chunk 2 (needed from row 5) is
            # emitted after row 1 so rows 0-1's PSUM copies aren't stuck
            # behind it.
            xcopies(0)
            xcopies(1)

            def mm_quads(h, pt, q0, q1):
                for q in range(q0, q1):
                    for g in range(NQ):
                        w = q * NQ + g
                        for ik in range(KH):
                            k = 97 if ik == 2 else 96
                            nc.tensor.matmul(
                                pt[32 * g : 32 * (g + 1), q * B : (q + 1) * B],
                                wslice(h, w, ik, k),
                                xslice(h + ik, w, k),
                                start=(ik == 0),
                                stop=(ik == 2),
                                tile_position=(0, 32 * g),
                            )

            outs = []  # (dram row, sbuf tile) deferred out DMAs
            NQW = W // NQ  # 16 quads per row
            for h in range(RPC):
                pt = ppool.tile([4 * OC, NQW * B], f32)
                ot = opool.tile([4 * OC, NQW * B], f16, tag=f"o{h}")
                if h == RPC - 1:
                    # copy per weight piece so the final out DMA's last
                    # dependency resolves right after the last quarter
                    for q0, q1 in WPIECES[h]:
                        mm_quads(h, pt, q0, q1)
                        nc.vector.tensor_copy(
                            ot[:, q0 * B : q1 * B], pt[:, q0 * B : q1 * B]
                        )
                else:
                    mm_quads(h, pt, 0, NQW)
                    nc.vector.tensor_copy(ot[:], pt[:])
                outs.append((out[h], ot))
                if h == 1:
                    xcopies(2)

            # out DMAs on the sync ring, issued after all weight DMAs so
            # weight transfers win the DMA-engine queue.
            for osl, ot in outs:
                nc.sync.dma_start(osl, ot[:])
    nc.compile()
    return nc


def _prep_inputs(x, weight, bias):
    """Host-side shard + layout prep.  Returns list of 8 per-core dicts."""
    # padded x, transposed to [c, hh, wp, b]
    xp = np.zeros((C, H + 2, W + 2, B), dtype=BF16)
    xp[:, 1 : H + 1, 1 : W + 1, :] = np.ascontiguousarray(
        x.transpose(1, 2, 3, 0)
    ).astype(BF16)

    # weight -> [h, j, c, w, ik, o], scaled into fp8e3 range
    wtr = np.ascontiguousarray(
        weight.transpose(0, 5, 3, 1, 4, 2) * np.float32(WSCALE)
    ).astype(F8E3)
    wtr = wtr.reshape(H, 96, W, KH, OC)
    btr = (bias.transpose(1, 2, 0) * np.float32(WSCALE)).astype(F8E3)  # [h,w,o]

    in_maps = []
    for i in range(NCORES):
        h0 = i * RPC
        xcore = np.ones((33, RPC + 2, WP, B), dtype=BF16)
        xcore[0:32] = xp[:, h0 : h0 + RPC + 2, :, :]

        wcore = np.zeros((RPC, 97, W, KH, OC), dtype=F8E3)
        wcore[:, 0:96] = wtr[h0 : h0 + RPC]
        wcore[:, 96, :, 2, :] = btr[h0 : h0 + RPC]  # bias via ones-row, ik==2

        in_maps.append(
            {
                "xs": np.ascontiguousarray(xcore.reshape(33, RPC + 2, WP * B)),
                "wt": np.ascontiguousarray(
                    wcore.reshape(RPC, 97, W * KH * OC)
                ),
            }
        )
    return in_maps


def _run(in_maps, trace=False, tmpdir=None):
    from concourse.bass_utils import run_bass_kernel_spmd

    if "nc" not in _cache:
        _cache["nc"] = _build_nc()
    return run_bass_kernel_spmd(
        _cache["nc"], in_maps, list(range(NCORES)), trace=trace, tmpdir=tmpdir
    )


def _assemble(results):
    out = np.empty((B, OC, H, W), dtype=np.float32)
    inv = np.float32(1.0 / WSCALE)
    for i in range(NCORES):
        # res: [h, g*32+o, q*16+b], w = q*4+g
        res = (
            results[i]["out"].astype(np.float32).reshape(RPC, NQ, OC, W // NQ, B)
            * inv
        )
        # -> out[b, o, h, q*4+g]
        out[:, :, i * RPC : (i + 1) * RPC, :] = res.transpose(
            4, 2, 0, 3, 1
        ).reshape(B, OC, RPC, W)
    return out


def kernel(x, weight, bias):
    x = np.asarray(x)
    weight = np.asarray(weight)
    bias = np.asarray(bias)
    in_maps = _prep_inputs(x, weight, bias)
    results = _run(in_maps).results
    return _assemble(results)


# revision 20
# speedup vs baseline: 1.1738x; 1.1156x over previous
"""LocallyConnected2d Bass kernel for 8 Trainium2 NeuronCores.

Problem (hardcoded): x[16,32,64,64] f32, weight[64,64,32,32,3,3] f32,
bias[32,64,64] f32 -> out[16,32,64,64] f32.  stride=1, pad=1, dil=1.

Sharding: outH split across 8 cores (8 rows each).  Per core, per output
row h: 64 w-positions x 3 kernel-rows of matmuls [K<=97,M=32]x[K,N=16]
accumulated in PSUM.  K = (kernel-col j)*32 + inC c, with a 97th "ones"
row carrying the bias.

Traffic optimizations vs the bf16 baseline:
  - weights (the dominant stream) stored fp8 e3m4, scaled by 2^8 on host
    (output descaled by 2^-8 on host - exact).  Halves weight HBM bytes.
  - x is DMAed once (unreplicated, [32c, 10hh, 66wp*16b] bf16) into
    partitions 0..31; the kernel-column-shifted copies for partition
    groups 1,2 (j=1,2) are made on-chip by DVE partition-shifted copies
    (4x perf mode), cutting x HBM bytes 3x.
  - one weight DMA per output row (row 7 in two halves to shorten the
    dependent tail); out DMAs issued on the sync ring after all weight
    DMAs so weights win the DMA-engine queue.

w-positions are processed in quads: position w = q*4+g is computed by a
matmul col-tiled to column group g (tile_position=(0,32g)), so the four
LDWEIGHTS+MATMUL streams of a quad run concurrently in the PE array.
PSUM tile is [128 = 4w x 32o, 16 quads x 16b] per output row.
"""

import numpy as np
import ml_dtypes

B, C, H, W = 16, 32, 64, 64
OC = 32
KH = KW = 3
NCORES = 8
RPC = H // NCORES  # rows per core = 8
NQ = 4  # quad size (PE col groups)
WSCALE = 2.0**8  # weight scale into fp8e3 range (max 15.08 < 15.5)
WP = W + 2  # padded width positions per row

BF16 = ml_dtypes.bfloat16
F8E3 = ml_dtypes.float8_e3m4

# x tile chunking by padded row hh: chunk -> (hh0, hh1)
XCHUNKS = [(0, 3), (3, 7), (7, 10)]

_cache = {}


def _build_nc():
    import concourse.bass as bass
    import concourse.tile as tile
    from concourse import bacc, mybir

    nc = bacc.Bacc(
        "TRN2", target_bir_lowering=False, debug=False, num_devices=NCORES
    )
    f32 = mybir.dt.float32
    f16 = mybir.dt.float16
    bf16 = mybir.dt.bfloat16
    f8e3 = mybir.dt.float8e3

    # xs: [33, 10, 66*16] bf16.  Partition c<32 holds x[c, hh, wp, b]
    # (hh = local padded row 0..9, wp = padded col 0..65, b = batch);
    # partition 32 is all-ones (bias row) so it rides the same DMA.
    xs = nc.dram_tensor("xs", (33, 10, WP * B), bf16, kind="ExternalInput")
    # wt: [8, 97, 64*3*32] f8e3, scaled by 2^8.  Partition map matches x:
    # 0..31 = (j=0, c), 32 = bias (nonzero only at ik==2 slots),
    # 33..64 = (j=1, c), 65..96 = (j=2, c).
    wt = nc.dram_tensor(
        "wt", (RPC, 97, W * KH * OC), f8e3, kind="ExternalInput"
    )
    # out: [8, 128, 16*16] f16 = 2^8 * out[h, g*32+o, q*16+b] with w = q*4+g
    out = nc.dram_tensor(
        "out", (RPC, 4 * OC, (W // NQ) * B), f16, kind="ExternalOutput"
    )

    with tile.TileContext(nc) as tc:
        with (
            tc.tile_pool(name="xpool", bufs=1) as xpool,
            tc.tile_pool(name="wpool", bufs=1) as wpool,
            tc.tile_pool(name="opool", bufs=1) as opool,
            tc.tile_pool(name="psum", bufs=3, space="PSUM") as ppool,
            tc.tile_pool(name="psum7", bufs=1, space="PSUM") as ppool7,
        ):
            # Per-chunk x tiles [97, rows, WP*16].  DMA x once into
            # partitions 0..31, ones row into 96; DVE makes the j=1,2
            # column-shifted copies into partitions 32..95.  Everything
            # rides the single sync ring in a hand-ordered sequence so
            # the serialized DMA engines see x0,x1,x2,w0,ones,w1,... and
            # the HWDGE descriptor-gen (~0.63us/DMA) stays ahead of the
            # transfer stream.
            xtiles = []
            for ci, (h0, h1) in enumerate(XCHUNKS):
                r = h1 - h0
                t = xpool.tile([97, r, WP * B], bf16, tag=f"x{ci}")
                nc.sync.dma_start(t[0:33], xs[:, h0:h1])
                xtiles.append(t)

            def xcopies(ci):
                t = xtiles[ci]
                for j in (1, 2):
                    nc.vector.tensor_copy(
                        t[32 * j + 1 : 32 * (j + 1) + 1, :, 0 : W * B],
                        t[0:32, :, j * B : j * B + W * B],
                    )

            def xslice(hh, w, k):
                for (h0, h1), t in zip(XCHUNKS, xtiles):
                    if h0 <= hh < h1:
                        return t[0:k, hh - h0, w * B : (w + 1) * B]
                raise AssertionError

            # Weight DMAs, one tile per row, loaded in quad-range pieces:
            # rows 0..3 whole, rows 4..6 in halves, row 7 in quarters.
            # Finer pieces toward the end shorten "weight bytes not yet
            # arrived when their dependent compute remains" without
            # letting the ~0.63us/DMA HWDGE cost outrun the transfers.
            WPIECES = {h: [(0, 16)] for h in range(4)}
            WPIECES.update({h: [(0, 8), (8, 16)] for h in (4, 5, 6)})
            WPIECES[7] = [(0, 4), (4, 8), (8, 12), (12, 16)]
            wtiles = {h: [] for h in range(RPC)}  # [(q0, q1, tile), ...]

            def load_w(h):
                for pi, (q0, q1) in enumerate(WPIECES[h]):
                    c0, c1 = q0 * NQ * KH * OC, q1 * NQ * KH * OC
                    t = wpool.tile([97, c1 - c0], f8e3, tag=f"w{h}_{pi}")
                    nc.sync.dma_start(t[:], wt[h, :, c0:c1])
                    wtiles[h].append((q0 * NQ, q1 * NQ, t))

            for h in range(RPC):
                load_w(h)

            def wslice(h, w, ik, k):
                for w0, w1, t in wtiles[h]:
                    if w0 <= w < w1:
                        return t[0:k, ((w - w0) * 3 + ik) * 32 :][:, 0:32]
                raise AssertionError

            # x replication copies for chunks 0,1 ahead of all PSUM
            # copies in the DVE queue; chunk 2 (needed from row 5) is
            # emitted after row 1 so rows 0-1's PSUM copies aren't stuck
            # behind it.
            xcopies(0)
            xcopies(1)

            def mm_quads(h, pt, q0, q1, pq0):
                for q in range(q0, q1):
                    for g in range(NQ):
                        w = q * NQ + g
                        for ik in range(KH):
                            nc.tensor.matmul(
                                pt[
                                    32 * g : 32 * (g + 1),
                                    (q - pq0) * B : (q - pq0 + 1) * B,
                                ],
                                wslice(h, w, ik, 97),
                                xslice(h + ik, w, 97),
                                start=(ik == 0),
                                stop=(ik == 2),
                                tile_position=(0, 32 * g),
                            )

            outs = []  # (dram row, sbuf tile) deferred out DMAs
            NQW = W // NQ  # 16 quads per row
            for h in range(RPC):
                ot = opool.tile([4 * OC, NQW * B], f16, tag=f"o{h}")
                if h == RPC - 1:
                    # separate PSUM tile + copy per weight piece (PSUM
                    # dependencies are tile-granular: sharing one tile
                    # would serialize piece k+1's matmuls on piece k's
                    # copy); one out DMA for the row.
                    for pi, (q0, q1) in enumerate(WPIECES[h]):
                        pt = ppool7.tile(
                            [4 * OC, (q1 - q0) * B], f32, tag=f"p7{pi}"
                        )
                        mm_quads(h, pt, q0, q1, q0)
                        nc.vector.tensor_copy(
                            ot[:, q0 * B : q1 * B], pt[:]
                        )
                else:
                    pt = ppool.tile([4 * OC, NQW * B], f32)
                    mm_quads(h, pt, 0, NQW, 0)
                    nc.vector.tensor_copy(ot[:], pt[:])
                outs.append((out[h], ot))
                if h == 1:
                    xcopies(2)

            # out DMAs on the sync ring, issued after all weight DMAs so
            # weight transfers win the DMA-engine queue.
            for osl, ot in outs:
                nc.sync.dma_start(osl, ot[:])
    nc.compile()
    return nc


def _prep_inputs(x, weight, bias):
    """Host-side shard + layout prep.  Returns list of 8 per-core dicts."""
    # padded x, transposed to [c, hh, wp, b]
    xp = np.zeros((C, H + 2, W + 2, B), dtype=BF16)
    xp[:, 1 : H + 1, 1 : W + 1, :] = np.ascontiguousarray(
        x.transpose(1, 2, 3, 0)
    ).astype(BF16)

    # weight -> [h, j, c, w, ik, o], scaled into fp8e3 range
    wtr = np.ascontiguousarray(
        weight.transpose(0, 5, 3, 1, 4, 2) * np.float32(WSCALE)
    ).astype(F8E3)
    wtr = wtr.reshape(H, 96, W, KH, OC)
    btr = (bias.transpose(1, 2, 0) * np.float32(WSCALE)).astype(F8E3)  # [h,w,o]

    in_maps = []
    for i in range(NCORES):
        h0 = i * RPC
        xcore = np.ones((33, RPC + 2, WP, B), dtype=BF16)
        xcore[0:32] = xp[:, h0 : h0 + RPC + 2, :, :]

        # partition map: 0..31 = (j=0,c), 32 = bias row, 33..64 = (j=1,c),
        # 65..96 = (j=2,c); bias slot nonzero only at ik==2.
        wcore = np.zeros((RPC, 97, W, KH, OC), dtype=F8E3)
        wcore[:, 0:32] = wtr[h0 : h0 + RPC, 0:32]
        wcore[:, 32, :, 2, :] = btr[h0 : h0 + RPC]
        wcore[:, 33:97] = wtr[h0 : h0 + RPC, 32:96]

        in_maps.append(
            {
                "xs": np.ascontiguousarray(xcore.reshape(33, RPC + 2, WP * B)),
                "wt": np.ascontiguousarray(
                    wcore.reshape(RPC, 97, W * KH * OC)
                ),
            }
        )
    return in_maps


def _run(in_maps, trace=False, tmpdir=None):
    from concourse.bass_utils import run_bass_kernel_spmd

    if "nc" not in _cache:
        _cache["nc"] = _build_nc()
    return run_bass_kernel_spmd(
        _cache["nc"], in_maps, list(range(NCORES)), trace=trace, tmpdir=tmpdir
    )


def _assemble(results):
    out = np.empty((B, OC, H, W), dtype=np.float32)
    inv = np.float32(1.0 / WSCALE)
    for i in range(NCORES):
        # res: [h, g*32+o, q*16+b], w = q*4+g
        res = (
            results[i]["out"].astype(np.float32).reshape(RPC, NQ, OC, W // NQ, B)
            * inv
        )
        # -> out[b, o, h, q*4+g]
        out[:, :, i * RPC : (i + 1) * RPC, :] = res.transpose(
            4, 2, 0, 3, 1
        ).reshape(B, OC, RPC, W)
    return out


def kernel(x, weight, bias):
    x = np.asarray(x)
    weight = np.asarray(weight)
    bias = np.asarray(bias)
    in_maps = _prep_inputs(x, weight, bias)
    results = _run(in_maps).results
    return _assemble(results)
